# revision 1
# baseline (speedup 1.0000x reference)
"""Multi-head self-attention on 8 Trainium2 NeuronCores.

Problem: x:(4,2048,1024) fp32; q = x@Wq, kv = x@Wkv (k,v split), 8 heads of
dim 64, softmax(q k^T / 8) v, concat heads, @Wo + bo -> (4,2048,1024).

Sharding: core c handles batch b=c//2 and head group g=c%2 (4 of 8 heads).
Each core computes its batch's projections restricted to its 4 heads, full
attention for those heads, and a partial output projection y_c = U_norm @ Wo_g.
Host gathers: out[b] = y_{2b} + y_{2b+1} + bo  (the "all-reduce" of the
tensor-parallel head split, done at unshard time).

Device algorithm (per core), all matmul operands fp16, PSUM accumulate fp32:
  - host supplies xT = x[b].T so the contraction dim (QDIM) is the partition
    axis; projections compute qT/kT (head_dim-major) and v (seq-major) tiles.
  - attention per head, per i-half (1024 q rows), per j-tile (128 k rows):
      simT[j,i] = kT_h(j)^T-tile @ qT_h        (PE, K=64)
      expT = exp(SCALE*simT)                   (ACT, reads PSUM directly)
      U~[d,i] += [v_h | 1]^T @ expT            (PE, K=128; row 64 = softmax sum)
    then normalization: r = 1/s via fast-reciprocal (DVE), R = ones x r
    broadcast (PE K=1 matmul), U_norm = U~ * R (DVE).
  - y[m,:] = U_norm_pairs^T @ Wo_g (K=128 per head pair), DVE drain, DMA out.
"""

import numpy as np

# ---- problem constants (hardcoded per the harness contract) ----
B, N, QDIM = 4, 2048, 1024
HEADS, DIM_MODEL = 8, 512
HEAD_DIM = DIM_MODEL // HEADS  # 64
SCALE = HEAD_DIM ** -0.5  # 0.125
N_CORES = 8
HEADS_PER_CORE = HEADS // 2  # 4 (head-group split across 2 cores per batch)
DMC = HEADS_PER_CORE * HEAD_DIM  # 256 per-core model dim slice


def build_nc(seq=N, qd=QDIM, nh=HEADS_PER_CORE, hd=HEAD_DIM, dout=QDIM,
             scale=SCALE, ihw=1024, skip_norm=False, norm_mode='dve',
             expp_bufs=3, upool_bufs=2, rows_bufs=2, ysb_bufs=3,
             phases='all', simp_bufs=2, uaccp_bufs=1, spare_bufs=2,
             xt_one_dma=True, y_pair_dma=True):
    """Build the per-core Bass program (same program on all 8 cores)."""
    from contextlib import ExitStack

    import concourse.bass as bass
    import concourse.tile as tile
    from concourse import bacc, mybir

    P = 128
    NC5 = 512  # psum bank width in fp32
    f16 = mybir.dt.float16
    f32 = mybir.dt.float32
    Exp = mybir.ActivationFunctionType.Exp
    Ln = mybir.ActivationFunctionType.Ln

    dmc = nh * hd                 # per-core projected dim (256)
    kt = qd // P                  # contraction tiles over QDIM (8)
    seqt = seq // P               # seq tiles (16)
    mtiles = max(1, dmc // P)     # qT/kT partition tiles (2)
    heads_per_mtile = nh // mtiles
    ihw = min(ihw, seq)           # i-half width
    n_ih = seq // ihw
    npairs = mtiles               # head pairs stacked for final proj (2)

    def chunks(total, w=NC5):
        c0 = 0
        while c0 < total:
            yield c0, min(w, total - c0)
            c0 += w

    nc = bacc.Bacc("TRN2", target_bir_lowering=False, debug=False,
                   num_devices=N_CORES)

    xt = nc.dram_tensor("xt", (qd, seq), f16, kind="ExternalInput").ap()
    wq = nc.dram_tensor("wq", (qd, dmc), f16, kind="ExternalInput").ap()
    wk = nc.dram_tensor("wk", (qd, dmc), f16, kind="ExternalInput").ap()
    wv = nc.dram_tensor("wv", (qd, dmc), f16, kind="ExternalInput").ap()
    wo = nc.dram_tensor("wo", (dmc, dout), f16, kind="ExternalInput").ap()
    y = nc.dram_tensor("y", (seq, dout), f32, kind="ExternalOutput").ap()

    with tile.TileContext(nc) as tc, ExitStack() as ctx:
        # ---- SBUF pools ----
        persist = ctx.enter_context(tc.tile_pool(name="persist", bufs=1))
        expp = ctx.enter_context(tc.tile_pool(name="expp", bufs=expp_bufs))
        upool = ctx.enter_context(tc.tile_pool(name="upool", bufs=upool_bufs))
        rows = ctx.enter_context(tc.tile_pool(name="rows", bufs=rows_bufs))
        ysb = ctx.enter_context(tc.tile_pool(name="ysb", bufs=ysb_bufs))
        # ---- PSUM pools (8 banks total: 2 spare + 4 sim + 2 uacc) ----
        spare = ctx.enter_context(tc.tile_pool(name="spare", bufs=spare_bufs, space="PSUM"))
        simp = ctx.enter_context(tc.tile_pool(name="simp", bufs=simp_bufs, space="PSUM"))
        uaccp = ctx.enter_context(tc.tile_pool(name="uaccp", bufs=uaccp_bufs, space="PSUM"))

        # ---- persistent SBUF tensors ----
        xt_sb = persist.tile([P, kt, seq], f16)
        wq_sb = persist.tile([P, kt, dmc], f16)
        wk_sb = persist.tile([P, kt, dmc], f16)
        wv_sb = persist.tile([P, kt, dmc], f16)
        wo_sb = persist.tile([min(P, dmc), npairs, dout], f16)
        v_sb = persist.tile([P, seqt, nh, hd + 1], f16)
        qt_sb = persist.tile([min(P, dmc), mtiles, seq], f16)
        kt_sb = persist.tile([min(P, dmc), mtiles, seq], f16)
        upairs = [persist.tile([min(P, dmc), seq], f16, name=f"upair{p}")
                  for p in range(npairs)]
        ones65 = persist.tile([65, hd], f16)

        # ---- input loads ----
        if xt_one_dma:
            nc.sync.dma_start(xt_sb[:], xt.rearrange("(ko ki) s -> ki ko s",
                                                     ki=P))
        else:
            for ko in range(kt):
                nc.sync.dma_start(xt_sb[:, ko, :], xt[ko * P:(ko + 1) * P, :])
        nc.sync.dma_start(wk_sb[:], wk.rearrange("(ko ki) m -> ki ko m", ki=P))
        nc.sync.dma_start(wq_sb[:], wq.rearrange("(ko ki) m -> ki ko m", ki=P))
        nc.sync.dma_start(wv_sb[:], wv.rearrange("(ko ki) m -> ki ko m", ki=P))
        nc.sync.dma_start(wo_sb[:], wo.rearrange("(t p) n -> p t n", p=min(P, dmc)))
        nc.vector.memset(v_sb[:, :, :, hd:hd + 1], 1.0)
        nc.vector.memset(ones65[:], 1.0)

        def proj_kq_tile(mt, which, n0, nw):
            """One [mp, nw] tile of kT (which=0) or qT (which=1) for m-tile mt."""
            mp = min(P, dmc)
            w_sb, out_sb = ((wk_sb, kt_sb), (wq_sb, qt_sb))[which]
            ps = spare.tile([mp, NC5], f32, tag="ps512", name="ps")
            for ko in range(kt):
                nc.tensor.matmul(
                    ps[:, 0:nw],
                    lhsT=w_sb[:, ko, mt * mp:(mt + 1) * mp],
                    rhs=xt_sb[:, ko, n0:n0 + nw],
                    start=(ko == 0), stop=(ko == kt - 1))
            nc.vector.tensor_copy(
                out_sb[0:mp, mt, n0:n0 + nw], ps[:, 0:nw])

        def proj_v_tile(jt):
            """v natural layout [seq, dmc] -> v_sb[:, jt, h, 0:hd]."""
            ps = spare.tile([P, dmc], f32, tag="ps512", name="ps")
            for ko in range(kt):
                nc.tensor.matmul(
                    ps[:],
                    lhsT=xt_sb[:, ko, jt * P:(jt + 1) * P],
                    rhs=wv_sb[:, ko, :],
                    start=(ko == 0), stop=(ko == kt - 1))
            nc.vector.tensor_copy(
                v_sb[:, jt, :, 0:hd],
                ps.rearrange("p (h d) -> p h d", h=nh))

        def attn_head_ih(h, ih, nm_override=None, pre_norm_cb=None):
            if True:
                nmode = nm_override or norm_mode
                mt = h // heads_per_mtile
                hb = (h % heads_per_mtile) * hd
                pair = h // heads_per_mtile
                i0 = ih * ihw
                uacc = uaccp.tile([hd + 1, ihw], f32, tag="uacc")
                for jt in range(seqt):
                    sim = simp.tile([P, ihw], f32, tag="sim")
                    for c0, cw in chunks(ihw):
                        nc.tensor.matmul(
                            sim[:, c0:c0 + cw],
                            lhsT=kt_sb[hb:hb + hd, mt, jt * P:(jt + 1) * P],
                            rhs=qt_sb[hb:hb + hd, mt, i0 + c0:i0 + c0 + cw],
                            start=True, stop=True)
                    expt = expp.tile([P, ihw], f16, tag="expt")
                    nc.scalar.activation(expt[:], sim[:], Exp, scale=scale)
                    for c0, cw in chunks(ihw):
                        nc.tensor.matmul(
                            uacc[:, c0:c0 + cw],
                            lhsT=v_sb[:, jt, h, :],
                            rhs=expt[:, c0:c0 + cw],
                            start=(jt == 0), stop=(jt == seqt - 1))
                if pre_norm_cb is not None:
                    # emit next phase's projection granules here so their DVE
                    # drains are ordered BEFORE this phase's norm chain
                    pre_norm_cb()
                # normalization: r = 1/s; U_norm = U~ * broadcast(r)
                u_sb = upool.tile([hd, ihw], f16, tag="u")
                nc.vector.tensor_copy(u_sb[:], uacc[0:hd, :])
                if skip_norm:
                    nc.vector.tensor_copy(
                        upairs[pair][hb:hb + hd, i0:i0 + ihw], u_sb[:])
                    return
                srow = rows.tile([65, ihw], f32, tag="srow")
                nc.vector.tensor_copy(srow[64:65, :], uacc[hd:hd + 1, :])
                # r = 1/s. The fused custom-DVE reciprocal op returns garbage
                # on this HW path, so either ACT ln/exp ("ln") or a manual
                # Newton iteration from standard DVE ops ("dve", default —
                # keeps the critical ACT engine free for the softmax exps).
                rrow = rows.tile([65, ihw], f32, tag="rrow")
                rrow16 = rows.tile([65, ihw], f16, tag="rrow16")
                if nmode == "ln":
                    lnrow = rows.tile([65, ihw], f32, tag="lnrow")
                    nc.scalar.activation(lnrow[64:65, :], srow[64:65, :], Ln)
                    nc.scalar.activation(rrow[64:65, :], lnrow[64:65, :], Exp,
                                         scale=-1.0)
                    nc.vector.tensor_copy(rrow16[64:65, :], rrow[64:65, :])
                elif nmode == "dve":
                    i32 = mybir.dt.int32
                    s_r, u_r, t_r = (srow[64:65, :], rrow[64:65, :],
                                     None)
                    trow = rows.tile([65, ihw], f32, tag="trow")
                    t_r = trow[64:65, :]
                    # u0 = bitcast(~bits(s)) * 0.23549792   (u = -1/s approx)
                    nc.vector.tensor_scalar(t_r.bitcast(i32), s_r.bitcast(i32),
                                            -1, None,
                                            op0=mybir.AluOpType.bitwise_xor)
                    nc.vector.tensor_scalar_mul(u_r, t_r, 0.23549792)
                    # two Newton passes: u <- (s*u + c)*u, c = 2.0017324, 2.0
                    for c in (2.0017324, 2.0):
                        nc.vector.tensor_mul(t_r, s_r, u_r)
                        nc.vector.scalar_tensor_tensor(
                            u_r, t_r, float(c), u_r,
                            op0=mybir.AluOpType.add, op1=mybir.AluOpType.mult)
                    # r = -u, cast to fp16
                    nc.vector.tensor_scalar_mul(rrow16[64:65, :], u_r, -1.0)
                elif nmode == "copy":  # timing-only bisect: wrong math
                    nc.vector.tensor_copy(rrow16[64:65, :], srow[64:65, :])
                else:
                    raise ValueError(nmode)
                for c0, cw in chunks(ihw):
                    rps = spare.tile([hd, NC5], f32, tag="ps512")
                    nc.tensor.matmul(
                        rps[:, 0:cw],
                        lhsT=ones65[64:65, :],
                        rhs=rrow16[64:65, c0:c0 + cw],
                        start=True, stop=True)
                    nc.vector.tensor_mul(
                        upairs[pair][hb:hb + hd, i0 + c0:i0 + c0 + cw],
                        u_sb[:, c0:c0 + cw], rps[:, 0:cw])

        def final_proj(ms=None):
            mp = min(P, dmc)
            for m in (range(seqt) if ms is None else ms):
                if y_pair_dma:
                    yt = ysb.tile([P, dout], f32, tag="yt")
                for n0, nw in chunks(dout):
                    yps = spare.tile([P, NC5], f32, tag="ps512")
                    for p in range(npairs):
                        nc.tensor.matmul(
                            yps[:, 0:nw],
                            lhsT=upairs[p][0:mp, m * P:(m + 1) * P],
                            rhs=wo_sb[0:mp, p, n0:n0 + nw],
                            start=(p == 0), stop=(p == npairs - 1))
                    if y_pair_dma:
                        nc.vector.tensor_copy(yt[:, n0:n0 + nw], yps[:, 0:nw])
                    else:
                        yt = ysb.tile([P, NC5], f32, tag="yt")
                        nc.vector.tensor_copy(yt[:, 0:nw], yps[:, 0:nw])
                        nc.sync.dma_start(
                            y[m * P:(m + 1) * P, n0:n0 + nw], yt[:, 0:nw])
                if y_pair_dma:
                    nc.sync.dma_start(y[m * P:(m + 1) * P, :], yt[:])

        # Emission schedule: per-engine instruction order is static after
        # scheduling, so projection granules are threaded between attention
        # (h, ih) phases — each phase's inputs emitted one phase ahead; the
        # ACT-paced attention then hides the remaining projection PE work.
        attn_phases = [(h, ih) for h in range(nh) for ih in range(n_ih)]

        def phase_needs(idx):
            # granules that must be emitted before attention phase idx;
            # every phase's j-loop consumes ALL v tiles, so v has deadline 0.
            if idx >= len(attn_phases):
                return []
            h, ih = attn_phases[idx]
            mt = h // heads_per_mtile
            need = [("k", mt, n0, nw) for n0, nw in chunks(seq)]
            need += [("q", mt, n0, nw) for n0, nw in chunks(seq)
                     if n0 < (ih + 1) * ihw and n0 + nw > ih * ihw]
            if idx == 0:
                need += [("v", jt) for jt in range(seqt)]
            return need

        emitted = set()

        def emit_granules(needs):
            for g in needs:
                if g in emitted:
                    continue
                emitted.add(g)
                if g[0] == "v":
                    proj_v_tile(g[1])
                else:
                    which = 0 if g[0] == "k" else 1
                    proj_kq_tile(g[1], which, g[2], g[3])

        all_granules = []
        for idx in range(len(attn_phases)):
            for g in phase_needs(idx):
                if g not in all_granules:
                    all_granules.append(g)

        if phases == 'proj':
            emit_granules(all_granules)
        else:
            emit_granules(phase_needs(0))
            # deadline-ordered backlog, spread evenly across early boundaries
            backlog = [g for g in all_granules if g not in emitted]
            nb = max(1, len(attn_phases) - 2)
            share = -(-len(backlog) // nb)
            last = len(attn_phases) - 1
            for idx, (h, ih) in enumerate(attn_phases):
                def _cb(idx=idx):
                    emit_granules(phase_needs(idx + 1))
                    take = [g for g in backlog if g not in emitted][:share]
                    emit_granules(take)
                attn_head_ih(h, ih, nm_override="ln" if idx == last else None,
                             pre_norm_cb=_cb)
                if phases == 'all' and idx == last - 1 and n_ih > 1:
                    # final-proj m-tiles whose i-range completes at the
                    # second-to-last phase overlap the last phase's attention
                    lh, lih = attn_phases[last]
                    done_ih = [p_ih for p_ih in range(n_ih) if p_ih != lih]
                    ms = [m for m in range(seqt)
                          if (m * P) // ihw in done_ih]
                    final_proj(ms)
            if phases == 'all':
                lh, lih = attn_phases[last]
                if n_ih > 1:
                    final_proj([m for m in range(seqt)
                                if (m * P) // ihw == lih])
                else:
                    final_proj()

    nc.compile()
    return nc


_NC_CACHE = {}


def _get_nc():
    if "nc" not in _NC_CACHE:
        _NC_CACHE["nc"] = build_nc()
    return _NC_CACHE["nc"]


def _prep_core_inputs(x, Wq, Wkv, Wo):
    """Host-side shard + layout prep: per-core fp16 slices."""
    f16 = np.float16
    in_maps = []
    for c in range(N_CORES):
        b, g = c // 2, c % 2
        s = slice(g * DMC, (g + 1) * DMC)
        in_maps.append({
            "xt": np.ascontiguousarray(x[b].T).astype(f16),
            "wq": np.ascontiguousarray(Wq[:, s]).astype(f16),
            "wk": np.ascontiguousarray(Wkv[:, g * DMC:(g + 1) * DMC]).astype(f16),
            "wv": np.ascontiguousarray(
                Wkv[:, DIM_MODEL + g * DMC:DIM_MODEL + (g + 1) * DMC]).astype(f16),
            "wo": np.ascontiguousarray(Wo[s, :]).astype(f16),
        })
    return in_maps


def kernel(x, Wq, Wkv, Wo, bo):
    from concourse import bass_utils

    x = np.asarray(x, dtype=np.float32)
    Wq = np.asarray(Wq, dtype=np.float32)
    Wkv = np.asarray(Wkv, dtype=np.float32)
    Wo = np.asarray(Wo, dtype=np.float32)
    bo = np.asarray(bo, dtype=np.float32)

    nc = _get_nc()
    in_maps = _prep_core_inputs(x, Wq, Wkv, Wo)
    res = bass_utils.run_bass_kernel_spmd(nc, in_maps,
                                          core_ids=list(range(N_CORES)))
    out = np.empty((B, N, QDIM), dtype=np.float32)
    for b in range(B):
        out[b] = res.results[2 * b]["y"] + res.results[2 * b + 1]["y"] + bo
    return out



# revision 38
# speedup vs baseline: 1.3670x; 1.3670x over previous
"""Multi-head self-attention on 8 Trainium2 NeuronCores.

Problem: x:(4,2048,1024) fp32; q = x@Wq, kv = x@Wkv (k,v split), 8 heads of
dim 64, softmax(q k^T / 8) v, concat heads, @Wo + bo -> (4,2048,1024).

Sharding: core c handles batch b=c//2 and head group g=c%2 (4 of 8 heads).
Host gathers: out[b] = y_{2b} + y_{2b+1} + bo (tensor-parallel head split
reduced at unshard time).

Device algorithm (per core), matmul operands fp16, PSUM accumulate fp32:
  - projections: qT/kT [dm, seq] (head-major) and v [seq, dm] tiles, with a
    ones column appended to v so the attn@v matmul also accumulates the
    softmax denominator.
  - attention in PAIRED windows: both heads of a head-pair share one
    [128, 1024] sim tile (512 i-columns each) and ONE exp instruction, so
    512-wide i-blocks still amortize the ACT access latency fully:
      simT[j, i]   = kT_h(j)-tile^T @ qT_h      (PE, K=64, out free = i)
      expT         = exp(SCALE*simT)            (ACT, PSUM -> SBUF fp16)
      U_h[i, 0:65] += expT_sub^T @ [v_h | 1]    (PE, out free = 65/queries
                                                 on partitions)
    then r = 1/s via DVE Newton; U_norm = U * r (per-partition scalar);
    U[i,d] tiles transposed back to [d,i] via the DMA XBAR (PE matmul
    transpose for the last window to skip the DMA latency on the tail).
  - y[m,:] = sum_p upairs[p](m)^T @ Wo_p, drains split DVE/ACT, DMA fp16.

Scheduling: pair-interleaved window order makes output-projection fill
work available evenly; projection granules are threaded between attention
j-steps in deadline order (the engine queues are strictly FIFO, so the
instruction that parks on the exp result is always emitted last in each
step). Dummy 0x0 matmuls warm the PE p-state ramp during the input DMAs.
"""

import numpy as np

# ---- problem constants (hardcoded per the harness contract) ----
B, N, QDIM = 4, 2048, 1024
HEADS, DIM_MODEL = 8, 512
HEAD_DIM = DIM_MODEL // HEADS  # 64
SCALE = HEAD_DIM ** -0.5  # 0.125
N_CORES = 8
HEADS_PER_CORE = HEADS // 2  # 4 (head-group split across 2 cores per batch)
DMC = HEADS_PER_CORE * HEAD_DIM  # 256 per-core model dim slice


def build_nc(seq=N, qd=QDIM, nh=HEADS_PER_CORE, hd=HEAD_DIM, dout=QDIM,
             scale=SCALE, pws=None, drip_ns=420.0, warm_n=70,
             expp_bufs=8, ysb_bufs=4, rows_bufs=2,
             simp_bufs=2, uaccp_bufs=2, spare_bufs=2):
    """Build the per-core Bass program (same program on all 8 cores)."""
    from contextlib import ExitStack

    import concourse.bass as bass
    import concourse.tile as tile
    from concourse import bacc, mybir
    from concourse.masks import make_identity

    P = 128
    f16 = mybir.dt.float16
    f32 = mybir.dt.float32
    i32 = mybir.dt.int32
    Exp = mybir.ActivationFunctionType.Exp
    AO = mybir.AluOpType

    dmc = nh * hd                 # per-core projected dim (256)
    kt = qd // P                  # contraction tiles over QDIM (8)
    seqt = seq // P               # seq tiles (16)
    npairs = 2                    # head pairs; pair p = heads (2p, 2p+1)
    BW = 512                      # i-block width per paired window

    # paired windows (pair, i0, iw): pair-interleaved so each 512-block's
    # output projection becomes available at an even cadence; the first
    # block is split so the exp stream starts before xt has fully landed.
    if pws is None:
        pws = [(0, 0, 512), (0, 512, 512), (0, 1024, 512), (1, 0, 512),
               (0, 1536, 512), (1, 512, 512), (1, 1024, 512), (1, 1536, 512)]

    # projection granule schedules (c0, cw): fine-grained at the front so
    # the first window's deps clear while xt is still streaming in.
    K_GRAN = [(0, 128), (128, 384), (512, 512), (1024, 512), (1536, 512)]
    Q_GRAN = {0: [(0, 256), (256, 256), (512, 512), (1024, 512), (1536, 512)],
              1: [(0, 512), (512, 512), (1024, 512), (1536, 512)]}

    nc = bacc.Bacc("TRN2", target_bir_lowering=False, debug=False,
                   num_devices=N_CORES)

    xt = nc.dram_tensor("xt", (qd, seq), f16, kind="ExternalInput").ap()
    wq = nc.dram_tensor("wq", (qd, dmc), f16, kind="ExternalInput").ap()
    wk = nc.dram_tensor("wk", (qd, dmc), f16, kind="ExternalInput").ap()
    wv = nc.dram_tensor("wv", (qd, dmc), f16, kind="ExternalInput").ap()
    wo = nc.dram_tensor("wo", (dmc, dout), f16, kind="ExternalInput").ap()
    y = nc.dram_tensor("y", (seq, dout), f16, kind="ExternalOutput").ap()

    with tile.TileContext(nc) as tc, ExitStack() as ctx:
        # ---- SBUF pools ----
        persist = ctx.enter_context(tc.tile_pool(name="persist", bufs=1))
        expp = ctx.enter_context(tc.tile_pool(name="expp", bufs=expp_bufs))
        rows = ctx.enter_context(tc.tile_pool(name="rows", bufs=rows_bufs))
        ysb = ctx.enter_context(tc.tile_pool(name="ysb", bufs=ysb_bufs))
        # ---- PSUM pools (8 banks: 4 sim + 3 uacc + 1 spare) ----
        simp = ctx.enter_context(tc.tile_pool(name="simp", bufs=simp_bufs,
                                              space="PSUM"))
        uaccp = ctx.enter_context(tc.tile_pool(name="uaccp", bufs=uaccp_bufs,
                                               space="PSUM"))
        spare = ctx.enter_context(tc.tile_pool(name="spare", bufs=spare_bufs,
                                               space="PSUM"))

        # ---- persistent SBUF tensors ----
        xt_sb = persist.tile([P, kt, seq], f16)
        wq_sb = persist.tile([P, kt, dmc], f16)
        wk_sb = persist.tile([P, kt, dmc], f16)
        wv_sb = persist.tile([P, kt, dmc], f16)
        wo_sb = persist.tile([P, npairs, dout], f16)
        v_sb = persist.tile([P, seqt, nh, hd + 1], f16)
        qt_sb = persist.tile([P, npairs, seq], f16)
        kt_sb = persist.tile([P, npairs, seq], f16)
        u_all = persist.tile([P, npairs, seqt, P], f16)
        upairs = [persist.tile([P, seq], f16, name=f"upair{p}")
                  for p in range(npairs)]
        zbias = persist.tile([P, 1], f32)
        ident = persist.tile([P, P], f16)
        zwarm = persist.tile([P, P], f16)

        # ---- PE warmup: the tensor engine p-state ramps to full clock only
        # after ~3us of continuous busy; burn 0x0 matmuls during the input
        # DMAs so the first real projections run at full speed ----
        nc.vector.memset(zwarm[:], 0.0)
        nc.vector.memset(zbias[:], 0.0)
        warm_ps = simp.tile([P, 1024], f32, tag="sim", name="warm_ps")
        for _ in range(warm_n):
            nc.tensor.matmul(warm_ps[:, 0:P], lhsT=zwarm[:], rhs=zwarm[:],
                             start=True, stop=True)

        # ---- input loads: weights + leading xt slivers first so the first
        # projection granules start early; remaining xt streams behind ----
        xtr = xt.rearrange("(ko ki) s -> ki ko s", ki=P)
        nc.sync.dma_start(wk_sb[:], wk.rearrange("(ko ki) m -> ki ko m", ki=P))
        nc.sync.dma_start(wq_sb[:], wq.rearrange("(ko ki) m -> ki ko m", ki=P))
        nc.sync.dma_start(xt_sb[:, :, 0:256], xtr[:, :, 0:256])
        nc.sync.dma_start(xt_sb[:, :, 256:512], xtr[:, :, 256:512])
        nc.sync.dma_start(wv_sb[:], wv.rearrange("(ko ki) m -> ki ko m", ki=P))
        for c0 in range(512, seq, 512):
            nc.sync.dma_start(xt_sb[:, :, c0:c0 + 512],
                              xtr[:, :, c0:c0 + 512])
        nc.sync.dma_start(wo_sb[:], wo.rearrange("(t p) n -> p t n", p=P))
        nc.vector.memset(v_sb[:, :, :, hd:hd + 1], 1.0)
        make_identity(nc, ident[:])

        # ---- projection / outproj granule emitters (dedup by key) ----
        emitted = set()
        forced = [0.0]

        def emit_kq(which, mt, c0, cw, ps_ap=None):
            """[128, cw] granule of kT (which='k') or qT into SBUF."""
            key = (which, mt, c0)
            if key in emitted:
                return
            emitted.add(key)
            w_sb, out_sb = ((wk_sb, kt_sb), (wq_sb, qt_sb))[which == 'q']
            if ps_ap is None:
                ps = spare.tile([P, 512], f32, tag="ps", name="ps")
                ps_ap = ps[:, 0:cw]
            if not in_drip[0]:
                forced[0] += cw * 3.4
            for ko in range(kt):
                nc.tensor.matmul(
                    ps_ap,
                    lhsT=w_sb[:, ko, mt * P:(mt + 1) * P],
                    rhs=xt_sb[:, ko, c0:c0 + cw],
                    start=(ko == 0), stop=(ko == kt - 1))
            nc.vector.tensor_copy(out_sb[:, mt, c0:c0 + cw], ps_ap)

        def need_k(mt, jt):
            for c0, cw in K_GRAN:
                if c0 < (jt + 1) * P and c0 + cw > jt * P:
                    emit_kq('k', mt, c0, cw)

        def need_q(mt, i0, iw):
            for c0, cw in Q_GRAN[mt]:
                if c0 < i0 + iw and c0 + cw > i0:
                    emit_kq('q', mt, c0, cw)

        def emit_v(jt):
            """v natural layout for 128 keys -> v_sb[:, jt, h, 0:hd]."""
            key = ('v', jt)
            if key in emitted:
                return
            emitted.add(key)
            if not in_drip[0]:
                forced[0] += 860
            ps = spare.tile([P, 512], f32, tag="ps")
            for ko in range(kt):
                nc.tensor.matmul(
                    ps[:, 0:dmc],
                    lhsT=xt_sb[:, ko, jt * P:(jt + 1) * P],
                    rhs=wv_sb[:, ko, :],
                    start=(ko == 0), stop=(ko == kt - 1))
            nc.vector.tensor_copy(
                v_sb[:, jt, :, 0:hd],
                ps[:, 0:dmc].rearrange("p (h d) -> p h d", h=nh))

        def emit_transpose(pair, itg):
            key = ('t', pair, itg)
            if key in emitted:
                return
            emitted.add(key)
            nc.sync.dma_start_transpose(
                upairs[pair][:, itg * P:(itg + 1) * P],
                u_all[:, pair, itg, :])

        def emit_pe_transpose(pair, itg):
            key = ('t', pair, itg)
            if key in emitted:
                return
            emitted.add(key)
            tp = spare.tile([P, 1024], f16, tag="ps", name="tp")
            nc.tensor.transpose(tp[:, 0:P], u_all[:, pair, itg, :], ident[:])
            dst = upairs[pair][:, itg * P:(itg + 1) * P]
            if itg % 2 == 0:
                nc.scalar.mul(dst, tp[:, 0:P], 1.0)
            else:
                nc.vector.tensor_copy(dst, tp[:, 0:P])

        yt_tiles = {}
        yh_tiles = {}

        def emit_oph0(m, n0):
            """pair-0 half of the output projection -> SBUF f32 stash."""
            key = ('h', m, n0)
            if key in emitted:
                return
            emitted.add(key)
            if m not in yh_tiles:
                yh_tiles[m] = persist.tile([P, dout], f32, name=f"yh{m}")
            ps = spare.tile([P, 512], f32, tag="ps", name="ps")
            nc.tensor.matmul(
                ps[:], lhsT=upairs[0][:, m * P:(m + 1) * P],
                rhs=wo_sb[:, 0, n0:n0 + 512], start=True, stop=True)
            nc.vector.tensor_copy(yh_tiles[m][:, n0:n0 + 512], ps[:])

        def emit_outproj(m, n0, full=False, use_act=False):
            """pair-1 half + stashed pair-0 half -> y chunk, DMA'd as soon
            as it drains; full=True computes both pairs directly (used at
            the epilogue where the stash round-trip would serialize DVE)."""
            key = ('o', m, n0)
            if key in emitted:
                return
            if not full:
                emit_oph0(m, n0)
            emitted.add(key)
            if m not in yt_tiles:
                yt_tiles[m] = ysb.tile([P, dout], f16, tag="yt", name="yt")
            yt = yt_tiles[m]
            yps = spare.tile([P, 512], f32, tag="ps", name="yps")
            if full:
                for p in range(npairs):
                    nc.tensor.matmul(
                        yps[:], lhsT=upairs[p][:, m * P:(m + 1) * P],
                        rhs=wo_sb[:, p, n0:n0 + 512],
                        start=(p == 0), stop=(p == npairs - 1))
                if use_act:
                    nc.scalar.mul(yt[:, n0:n0 + 512], yps[:], 1.0)
                else:
                    nc.vector.tensor_copy(yt[:, n0:n0 + 512], yps[:])
            else:
                nc.tensor.matmul(
                    yps[:], lhsT=upairs[1][:, m * P:(m + 1) * P],
                    rhs=wo_sb[:, 1, n0:n0 + 512], start=True, stop=True)
                nc.vector.tensor_add(yt[:, n0:n0 + 512], yps[:],
                                     yh_tiles[m][:, n0:n0 + 512])
            nc.sync.dma_start(y[m * P:(m + 1) * P, n0:n0 + 512],
                              yt[:, n0:n0 + 512])

        # ---- deadline-ordered fill queue (FIFO; emitters dedup, so items
        # force-emitted at their use site pop later as free no-ops) ----
        fill_q = []
        carry = [0.0]
        cur_si = [0]
        in_drip = [False]

        def drip(budget_ns, cap=2100.0):
            carry[0] = min(carry[0] + budget_ns, cap)
            in_drip[0] = True
            # anything at-or-past its deadline (and everything queued before
            # it) is emitted unconditionally
            due = -1
            for i, (est, key, fn, min_si, deadline) in enumerate(fill_q):
                if key not in emitted and cur_si[0] >= deadline:
                    due = i
            if due >= 0:
                for est, key, fn, min_si, deadline in fill_q[:due + 1]:
                    if key not in emitted:
                        fn()
                del fill_q[:due + 1]
            while fill_q:
                est, key, fn, min_si, deadline = fill_q[0]
                if key in emitted:
                    fill_q.pop(0)
                    continue
                if est > carry[0] or min_si > cur_si[0]:
                    break
                fill_q.pop(0)
                fn()
                carry[0] -= est
            in_drip[0] = False

        def q_kq(which, mt, c0, cw):
            fill_q.append((int(cw * 3.4), (which, mt, c0),
                           lambda: emit_kq(which, mt, c0, cw), 0, 10 ** 9))

        # static backlog in deadline order: PW0's j-progressive needs first,
        # then later windows' k/q granules in first-use order.
        for jt in range(seqt):
            for c0, cw in K_GRAN:
                if jt and c0 == jt * P:
                    q_kq('k', 0, c0, cw)
            fill_q.append((860, ('v', jt), lambda jt=jt: emit_v(jt),
                           0, 10 ** 9))
        def q_q(mt, i0, iw):
            for c0, cw in Q_GRAN[mt]:
                if c0 < i0 + iw and c0 + cw > i0:
                    q_kq('q', mt, c0, cw)

        q_q(0, 256, 256)
        q_q(0, 512, 512)
        q_q(0, 1024, 512)
        for c0, cw in K_GRAN:
            q_kq('k', 1, c0, cw)
        q_q(1, 0, 512)
        q_q(0, 1536, 512)
        q_q(1, 512, 512)
        q_q(1, 1024, 512)
        q_q(1, 1536, 512)

        # ---- paired attention windows ----
        cover = {0: {}, 1: {}}

        class PW:
            def __init__(self, pair, i0, bw=BW):
                self.pair, self.i0, self.bw = pair, i0, bw
                self.ua = self.ub = None

        def emit_sim(w, jt):
            need_k(w.pair, jt)
            need_q(w.pair, w.i0, w.bw)
            sim = simp.tile([P, 1024], f32, tag="sim", name="sim")
            shared_bank = w.bw < 512
            for hl in range(2):
                nc.tensor.matmul(
                    sim[:, hl * w.bw:(hl + 1) * w.bw],
                    lhsT=kt_sb[hl * hd:(hl + 1) * hd, w.pair,
                               jt * P:(jt + 1) * P],
                    rhs=qt_sb[hl * hd:(hl + 1) * hd, w.pair,
                              w.i0:w.i0 + w.bw],
                    start=(hl == 0 or not shared_bank),
                    stop=(hl == 1 or not shared_bank))
            return sim

        def emit_attnv(w, jt, expt):
            if w.ua is None:
                w.ua = uaccp.tile([P, 4, hd + 1], f32, tag="uacc", name="ua")
                w.ub = uaccp.tile([P, 4, hd + 1], f32, tag="uacc", name="ub")
            nsub = w.bw // P
            for hl in range(2):
                t = (w.ua, w.ub)[hl]
                for isub in range(nsub):
                    nc.tensor.matmul(
                        t[:, isub, :],
                        lhsT=expt[:, hl * w.bw + isub * P:
                                  hl * w.bw + (isub + 1) * P],
                        rhs=v_sb[:, jt, 2 * w.pair + hl, :],
                        start=(jt == 0 and isub == 0),
                        stop=(jt == seqt - 1 and isub == nsub - 1))

        def window_end(w, last):
            # normalization for both heads: r = 1/s, U_norm = U * r.
            # DVE Newton in steady state; ACT ln/exp on the tail (shorter
            # serial chain, and ACT is idle once the exp stream ends).
            s8 = rows.tile([P, 8], f32, tag="s8")
            u8 = rows.tile([P, 8], f32, tag="u8")
            t8 = rows.tile([P, 8], f32, tag="t8")
            r32 = rows.tile([P, 8], f32, tag="r32")
            nsub = w.bw // P
            ns2 = 2 * nsub
            nc.vector.tensor_copy(s8[:, 0:nsub], w.ua[:, 0:nsub, hd])
            nc.vector.tensor_copy(s8[:, nsub:ns2], w.ub[:, 0:nsub, hd])
            # u0 = bitcast(~bits(s)) * 0.23549792   (u = -1/s approx)
            nc.vector.tensor_scalar(t8.bitcast(i32)[:, 0:ns2],
                                    s8.bitcast(i32)[:, 0:ns2],
                                    -1, None, op0=AO.bitwise_xor)
            nc.vector.tensor_scalar_mul(u8[:, 0:ns2], t8[:, 0:ns2],
                                        0.23549792)
            # two Newton passes: u <- (s*u + c)*u, c = 2.0017324, 2.0
            for c in (2.0017324, 2.0):
                nc.vector.tensor_mul(t8[:, 0:ns2], s8[:, 0:ns2], u8[:, 0:ns2])
                nc.vector.scalar_tensor_tensor(
                    u8[:, 0:ns2], t8[:, 0:ns2], float(c), u8[:, 0:ns2],
                    op0=AO.add, op1=AO.mult)
            nc.vector.tensor_scalar_mul(r32[:, 0:ns2], u8[:, 0:ns2], -1.0)

            def one_mul(hl, isub, on_act):
                itg = w.i0 // P + isub
                t = (w.ua, w.ub)[hl]
                r_ap = r32[:, nsub * hl + isub:nsub * hl + isub + 1]
                dst = u_all[:, w.pair, itg, hl * hd:(hl + 1) * hd]
                if on_act:
                    nc.scalar.mul(dst, t[:, isub, 0:hd], r_ap)
                else:
                    nc.vector.tensor_scalar(dst, t[:, isub, 0:hd],
                                            r_ap, None, op0=AO.mult)

            if last:
                # per-i-tile interleave: muls -> transpose -> outproj so the
                # engines pipeline down the tail
                for isub in range(nsub):
                    itg = w.i0 // P + isub
                    one_mul(0, isub, True)
                    one_mul(1, isub, False)
                    emit_pe_transpose(w.pair, itg)
                    for ci, n0 in enumerate(range(0, dout, 512)):
                        emit_outproj(itg, n0, full=True,
                                     use_act=(ci + isub) % 2 == 0)
            else:
                for hl in range(2):
                    for isub in range(nsub):
                        one_mul(hl, isub, False)
            w.ua = w.ub = None
            # pair complete for this i-range -> transposes now; pair-0 also
            # unlocks its outproj half; when both pairs cover a 512-block
            # the pair-1 half + add closes it
            last_blk = pws[-1][1] // BW
            for isub in range(nsub):
                itg = w.i0 // P + isub
                if last:
                    emit_pe_transpose(w.pair, itg)
                else:
                    fill_q.append((60, ('t', w.pair, itg),
                                   lambda p=w.pair, it=itg:
                                   emit_transpose(p, it), 0, 10 ** 9))
            if w.pair == 0:
                for isub in range(nsub):
                    m = w.i0 // P + isub
                    if m // (BW // P) == last_blk:
                        continue
                    for n0 in range(0, dout, 512):
                        fill_q.append((500, ('h', m, n0),
                                       lambda m=m, n0=n0:
                                       emit_oph0(m, n0),
                                       cur_si[0] + 4, 10 ** 9))
            for blk in range(w.i0 // BW, (w.i0 + w.bw - 1) // BW + 1):
                cov = cover[w.pair]
                cov[blk] = cov.get(blk, 0) + w.bw
                if cov[blk] >= BW and cover[1 - w.pair].get(blk, 0) >= BW:
                    for isub in range(BW // P):
                        m = blk * (BW // P) + isub
                        for n0 in range(0, dout, 512):
                            fill_q.append((500, ('o', m, n0),
                                           lambda m=m, n0=n0:
                                           emit_outproj(m, n0),
                                           cur_si[0] + 4, 10 ** 9))
            if last:
                # epilogue: flush; alternate PSUM drains DVE/ACT (ACT idles
                # once the exp stream ends)
                while fill_q:
                    est, key, fn, _, _ = fill_q.pop(0)
                    if key not in emitted:
                        fn()

        # ---- prologue: first k/q granules through a borrowed sim slot so
        # the single spare bank doesn't serialize the start ----
        pro = simp.tile([P, 1024], f32, tag="sim")
        emit_kq('k', 0, 0, 128, ps_ap=pro[:, 0:128])
        emit_kq('q', 0, 0, 256, ps_ap=pro[:, 512:768])

        wins = [PW(*t) for t in pws]
        steps = [(w, jt) for w in wins for jt in range(seqt)]
        cur_sim = emit_sim(*steps[0])
        for si, (w, jt) in enumerate(steps):
            cur_si[0] = si
            expt = expp.tile([P, 1024], f16, tag="expt", name="expt")
            nc.scalar.activation(expt[:, 0:2 * w.bw], cur_sim[:, 0:2 * w.bw],
                                 Exp, bias=zbias[:, 0:1], scale=scale)
            if si + 1 < len(steps):
                cur_sim = emit_sim(*steps[si + 1])
            emit_v(jt)
            drip(max(0.0, drip_ns - forced[0]))
            forced[0] = 0.0
            emit_attnv(w, jt, expt)
            if jt == seqt - 1:
                window_end(w, last=(si == len(steps) - 1))

    nc.compile()
    return nc


_NC_CACHE = {}


def _get_nc():
    if "nc" not in _NC_CACHE:
        _NC_CACHE["nc"] = build_nc()
    return _NC_CACHE["nc"]


def _prep_core_inputs(x, Wq, Wkv, Wo):
    """Host-side shard + layout prep: per-core fp16 slices."""
    f16 = np.float16
    in_maps = []
    for c in range(N_CORES):
        b, g = c // 2, c % 2
        s = slice(g * DMC, (g + 1) * DMC)
        in_maps.append({
            "xt": np.ascontiguousarray(x[b].T).astype(f16),
            "wq": np.ascontiguousarray(Wq[:, s]).astype(f16),
            "wk": np.ascontiguousarray(Wkv[:, g * DMC:(g + 1) * DMC]).astype(f16),
            "wv": np.ascontiguousarray(
                Wkv[:, DIM_MODEL + g * DMC:DIM_MODEL + (g + 1) * DMC]).astype(f16),
            "wo": np.ascontiguousarray(Wo[s, :]).astype(f16),
        })
    return in_maps


def kernel(x, Wq, Wkv, Wo, bo):
    from concourse import bass_utils

    x = np.asarray(x, dtype=np.float32)
    Wq = np.asarray(Wq, dtype=np.float32)
    Wkv = np.asarray(Wkv, dtype=np.float32)
    Wo = np.asarray(Wo, dtype=np.float32)
    bo = np.asarray(bo, dtype=np.float32)

    nc = _get_nc()
    in_maps = _prep_core_inputs(x, Wq, Wkv, Wo)
    res = bass_utils.run_bass_kernel_spmd(nc, in_maps,
                                          core_ids=list(range(N_CORES)))
    out = np.empty((B, N, QDIM), dtype=np.float32)
    for b in range(B):
        out[b] = (res.results[2 * b]["y"].astype(np.float32)
                  + res.results[2 * b + 1]["y"].astype(np.float32) + bo)
    return out
